# revision 12
# baseline (speedup 1.0000x reference)
"""Trainium2 Bass kernel for nn_CrossModalRouter (MoE routing).

Computation (matches the reference):
  gf      = mean(x, axis=(2,3))            # [B, C]  -- the heavy, memory-bound part
  hidden  = silu(gf @ W1 + b1)
  logits  = hidden @ W2 + b2               # [B, E]
  noisy   = logits + 2*noise
  top2    = top_k(noisy, 2) -> indices
  weights = softmax(logits[top2])
  counts  = bincount(indices, length=E)

Sharding: data-parallel over batch across 8 NeuronCores (8 batches/core).
Router weights replicated. Per-core partial bincounts summed on host
(the "all-reduce" for the stats buffer).

Device layout notes:
  x shard [8, 1024, 64, 64] is streamed as 32 tiles of [128p, 8192] --
  each tile is a fully contiguous 4 MiB DRAM region (channels 256j..256j+255
  of one batch; partition p holds channels 2p and 2p+1 of the block, i.e. an
  interleaved channel order). DVE reduce_sum produces per-channel sums into
  gft [128, 64] with column = k*8 + b where k = channel-block 0..7.
  W1 rows are permuted on the host into the same interleaved order (and
  pre-scaled by 1/4096 to fold in the mean), so PE matmuls contract
  correctly: hT = W1p.T @ gft_chunks -> silu -> logits = hT.T @ W2.
"""

import numpy as np
from contextlib import ExitStack

import concourse.bass as bass
import concourse.tile as tile
from concourse import bacc, mybir
from concourse.bass_utils import run_bass_kernel_spmd

F32 = mybir.dt.float32

B, C, H, W = 64, 1024, 64, 64
S = H * W                 # 4096 spatial
E = 8                     # experts
MID = 64                  # router hidden dim
NCORES = 8
BL = B // NCORES          # 8 batches per core
CK = 8                    # 128-channel blocks per batch
# XBUFS must equal the number of HWDGE sem lanes (8) so that a slot's
# previous writer sits on the same DMA lane as its next writer -- keeps
# each DMA at <=2 sync waits (walrus HWDGE limit).
XBUFS = 8

_CACHE = {}


def _build_program():
    # Bacc (not raw Bass): its compile() splits multi-sem sync waits into
    # event semaphores -- TRN2 allows only 1 wait per instruction.
    nc = bacc.Bacc("TRN2", target_bir_lowering=False, debug=False,
                   num_devices=NCORES)

    x_d = nc.dram_tensor("x", [BL, C, H, W], F32, kind="ExternalInput")
    w1_d = nc.dram_tensor("w1p", [8, 128, MID], F32, kind="ExternalInput")
    b1_d = nc.dram_tensor("b1c", [MID, 1], F32, kind="ExternalInput")
    w2_d = nc.dram_tensor("w2c", [MID, E], F32, kind="ExternalInput")
    b2_d = nc.dram_tensor("b2r", [BL, E], F32, kind="ExternalInput")
    bn_d = nc.dram_tensor("bn", [BL, E], F32, kind="ExternalInput")
    io_d = nc.dram_tensor("iota8", [BL, E], F32, kind="ExternalInput")
    on_d = nc.dram_tensor("ones8", [E, 1], F32, kind="ExternalInput")
    out_d = nc.dram_tensor("out", [BL, 5], F32, kind="ExternalOutput")

    # [b, k, p, (h w)]: channel c = k*128 + p; 16 KiB contiguous per
    # partition, 2 MiB contiguous per (b, k) tile.
    xv = x_d.ap().rearrange("b (k p) h w -> b k p (h w)", k=CK, p=128)

    AF = mybir.ActivationFunctionType
    OP = mybir.AluOpType

    with tile.TileContext(nc) as tc, ExitStack() as ctx:
        xpool = ctx.enter_context(tc.tile_pool(name="xp", bufs=XBUFS))
        cpool = ctx.enter_context(tc.tile_pool(name="cp", bufs=1))
        ppool = ctx.enter_context(
            tc.tile_pool(name="pp", bufs=1, space=bass.MemorySpace.PSUM))

        # --- constants / small inputs (SWDGE so they don't perturb the
        # HWDGE lane rotation used by the big x loads) ---
        w1sb = cpool.tile([128, 8, MID], F32)
        nc.gpsimd.dma_start(w1sb[:], w1_d.ap().rearrange("k p m -> p k m"))
        b1sb = cpool.tile([MID, 1], F32)
        nc.gpsimd.dma_start(b1sb[:], b1_d.ap())
        w2sb = cpool.tile([MID, E], F32)
        nc.gpsimd.dma_start(w2sb[:], w2_d.ap())
        b2sb = cpool.tile([BL, E], F32)
        nc.gpsimd.dma_start(b2sb[:], b2_d.ap())
        bnsb = cpool.tile([BL, E], F32)
        nc.gpsimd.dma_start(bnsb[:], bn_d.ap())
        iosb = cpool.tile([BL, E], F32)
        nc.gpsimd.dma_start(iosb[:], io_d.ap())
        onsb = cpool.tile([E, 1], F32)
        nc.gpsimd.dma_start(onsb[:], on_d.ap())

        # --- global average pool (as sums; 1/4096 folded into W1) ---
        # gft[p, k*8 + b] = sum_s x[b, k*128 + p, s]
        gft = cpool.tile([128, 8 * BL], F32)
        for b in range(BL):
            for k in range(CK):
                xt = xpool.tile([128, S], F32)
                nc.gpsimd.dma_start(xt[:], xv[b, k])
                nc.vector.reduce_sum(
                    gft[:, k * BL + b:k * BL + b + 1],
                    xt[:],
                    axis=mybir.AxisListType.X,
                )

        # --- router MLP ---
        # hT [MID, BL] = W1p.T @ gft  (accumulate over the 8 K-chunks of 128)
        psum_h = ppool.tile([MID, BL], F32)
        for k in range(8):
            nc.tensor.matmul(psum_h[:], w1sb[:, k, :],
                             gft[:, k * BL:(k + 1) * BL],
                             start=(k == 0), stop=(k == 7))
        # silu(z) = z * sigmoid(z), z = psum_h + b1  (Silu ACT table is not
        # implemented in the simulator, so build it from Sigmoid)
        hpre = cpool.tile([MID, BL], F32)
        nc.scalar.activation(hpre[:], psum_h[:], AF.Identity, bias=b1sb[:],
                             scale=1.0)
        hsig = cpool.tile([MID, BL], F32)
        nc.scalar.activation(hsig[:], hpre[:], AF.Sigmoid)
        hsb = cpool.tile([MID, BL], F32)
        nc.vector.tensor_tensor(hsb[:], hpre[:], hsig[:], op=OP.mult)

        # logits [BL, E] = (hT).T @ W2
        psum_l = ppool.tile([BL, E], F32)
        nc.tensor.matmul(psum_l[:], hsb[:], w2sb[:], start=True, stop=True)

        clean = cpool.tile([BL, E], F32)
        nc.vector.tensor_tensor(clean[:], psum_l[:], b2sb[:], op=OP.add)
        noisy = cpool.tile([BL, E], F32)
        nc.vector.tensor_tensor(noisy[:], psum_l[:], bnsb[:], op=OP.add)

        # --- top-2 ---
        m8 = cpool.tile([BL, 8], F32)
        nc.vector.max(m8[:], noisy[:])
        eq0 = cpool.tile([BL, E], F32)
        nc.vector.tensor_scalar(eq0[:], noisy[:], m8[:, 0:1], None,
                                op0=OP.is_equal)
        eq1 = cpool.tile([BL, E], F32)
        nc.vector.tensor_scalar(eq1[:], noisy[:], m8[:, 1:2], None,
                                op0=OP.is_equal)

        # (InstTensorTensorReduce crashes the exec unit on this runtime --
        # use separate mult + reduce_sum instead.)
        outsb = cpool.tile([BL, 5], F32)
        jj0 = cpool.tile([BL, E], F32)
        nc.vector.tensor_tensor(jj0[:], eq0[:], iosb[:], op=OP.mult)
        nc.vector.reduce_sum(outsb[:, 2:3], jj0[:], axis=mybir.AxisListType.X)
        jj1 = cpool.tile([BL, E], F32)
        nc.vector.tensor_tensor(jj1[:], eq1[:], iosb[:], op=OP.mult)
        nc.vector.reduce_sum(outsb[:, 3:4], jj1[:], axis=mybir.AxisListType.X)

        l0 = cpool.tile([BL, 1], F32)
        jj2 = cpool.tile([BL, E], F32)
        nc.vector.tensor_tensor(jj2[:], eq0[:], clean[:], op=OP.mult)
        nc.vector.reduce_sum(l0[:], jj2[:], axis=mybir.AxisListType.X)
        l1 = cpool.tile([BL, 1], F32)
        jj3 = cpool.tile([BL, E], F32)
        nc.vector.tensor_tensor(jj3[:], eq1[:], clean[:], op=OP.mult)
        nc.vector.reduce_sum(l1[:], jj3[:], axis=mybir.AxisListType.X)

        # --- expert counts: onehot.T @ ones ---
        onehot = cpool.tile([BL, E], F32)
        nc.vector.tensor_tensor(onehot[:], eq0[:], eq1[:], op=OP.add)
        psum_c = ppool.tile([E, 1], F32)
        nc.tensor.matmul(psum_c[:], onehot[:], onsb[:], start=True, stop=True)

        # --- softmax over the two selected clean logits ---
        mx = cpool.tile([BL, 1], F32)
        nc.vector.tensor_tensor(mx[:], l0[:], l1[:], op=OP.max)
        nmx = cpool.tile([BL, 1], F32)
        nc.vector.tensor_scalar_mul(nmx[:], mx[:], -1.0)
        ex0 = cpool.tile([BL, 1], F32)
        nc.scalar.activation(ex0[:], l0[:], AF.Exp, bias=nmx[:], scale=1.0)
        ex1 = cpool.tile([BL, 1], F32)
        nc.scalar.activation(ex1[:], l1[:], AF.Exp, bias=nmx[:], scale=1.0)
        ssum = cpool.tile([BL, 1], F32)
        nc.vector.tensor_tensor(ssum[:], ex0[:], ex1[:], op=OP.add)
        rcp = cpool.tile([BL, 1], F32)
        nc.vector.reciprocal(rcp[:], ssum[:])
        nc.vector.tensor_tensor(outsb[:, 0:1], ex0[:], rcp[:], op=OP.mult)
        nc.vector.tensor_tensor(outsb[:, 1:2], ex1[:], rcp[:], op=OP.mult)
        # DVE (not ACT) so the out DMA only waits on one compute engine.
        nc.vector.tensor_copy(outsb[:, 4:5], psum_c[:])

        nc.gpsimd.dma_start(out_d.ap(), outsb[:])

    nc.compile()
    return nc


def _get_program():
    if "nc" not in _CACHE:
        _CACHE["nc"] = _build_program()
    return _CACHE["nc"]


def _host_inputs(x, noise, W1, b1, W2, b2):
    x = np.ascontiguousarray(np.asarray(x, dtype=np.float32))
    noise = np.asarray(noise, dtype=np.float32)
    W1 = np.asarray(W1, dtype=np.float32)
    b1 = np.asarray(b1, dtype=np.float32)
    W2 = np.asarray(W2, dtype=np.float32)
    b2 = np.asarray(b2, dtype=np.float32)

    # W1 split into the 8 K-chunks of 128 channels, pre-scaled by 1/S to
    # turn spatial sums into means.
    w1p = np.ascontiguousarray((W1 / np.float32(S)).reshape(8, 128, MID))

    b1c = np.ascontiguousarray(b1.reshape(MID, 1))
    w2c = np.ascontiguousarray(W2)
    b2r = np.ascontiguousarray(np.broadcast_to(b2, (BL, E)))
    iota8 = np.ascontiguousarray(
        np.broadcast_to(np.arange(E, dtype=np.float32), (BL, E)))
    ones8 = np.ones((E, 1), dtype=np.float32)

    in_maps = []
    for c in range(NCORES):
        sl = slice(c * BL, (c + 1) * BL)
        in_maps.append({
            "x": x[sl],
            "w1p": w1p,
            "b1c": b1c,
            "w2c": w2c,
            "b2r": b2r,
            "bn": np.ascontiguousarray(b2r + 2.0 * noise[sl]),
            "iota8": iota8,
            "ones8": ones8,
        })
    return in_maps


def run(x, noise, W1, b1, W2, b2, trace=False, **spmd_kwargs):
    """Run the device kernel; returns (weights, indices, counts) plus the
    BassKernelResults (for profiling from a test harness)."""
    nc = _get_program()
    in_maps = _host_inputs(x, noise, W1, b1, W2, b2)
    res = run_bass_kernel_spmd(nc, in_maps, list(range(NCORES)),
                               trace=trace, **spmd_kwargs)
    outs = [res.results[c]["out"] for c in range(NCORES)]
    weights = np.concatenate([o[:, 0:2] for o in outs], axis=0)
    weights = np.ascontiguousarray(weights, dtype=np.float32)
    indices = np.rint(
        np.concatenate([o[:, 2:4] for o in outs], axis=0)).astype(np.int32)
    counts = np.rint(
        np.sum([o[:, 4] for o in outs], axis=0)).astype(np.int32)
    return (weights, indices, counts), res


def kernel(x, noise, W1, b1, W2, b2):
    (weights, indices, counts), _ = run(x, noise, W1, b1, W2, b2)
    return weights, indices, counts


# revision 14
# speedup vs baseline: 2.9158x; 2.9158x over previous
"""Trainium2 Bass kernel for nn_CrossModalRouter (MoE routing).

Computation (matches the reference):
  gf      = mean(x, axis=(2,3))            # [B, C]  -- the heavy, memory-bound part
  hidden  = silu(gf @ W1 + b1)
  logits  = hidden @ W2 + b2               # [B, E]
  noisy   = logits + 2*noise
  top2    = top_k(noisy, 2) -> indices
  weights = softmax(logits[top2])
  counts  = bincount(indices, length=E)

Sharding: data-parallel over batch across 8 NeuronCores (8 batches/core).
Router weights replicated. Per-core partial bincounts summed on host
(the "all-reduce" for the stats buffer).

Device layout notes:
  x shard [8, 1024, 64, 64] is streamed as 32 tiles of [128p, 8192] --
  each tile is a fully contiguous 4 MiB DRAM region (channels 256j..256j+255
  of one batch; partition p holds channels 2p and 2p+1 of the block, i.e. an
  interleaved channel order). DVE reduce_sum produces per-channel sums into
  gft [128, 64] with column = k*8 + b where k = channel-block 0..7.
  W1 rows are permuted on the host into the same interleaved order (and
  pre-scaled by 1/4096 to fold in the mean), so PE matmuls contract
  correctly: hT = W1p.T @ gft_chunks -> silu -> logits = hT.T @ W2.
"""

import numpy as np
from contextlib import ExitStack

import concourse.bass as bass
import concourse.tile as tile
from concourse import bacc, mybir
from concourse.bass_utils import run_bass_kernel_spmd

F32 = mybir.dt.float32

B, C, H, W = 64, 1024, 64, 64
S = H * W                 # 4096 spatial
E = 8                     # experts
MID = 64                  # router hidden dim
NCORES = 8
BL = B // NCORES          # 8 batches per core
CK = 8                    # 128-channel blocks per batch
# XBUFS must equal the number of HWDGE sem lanes (8) so that a slot's
# previous writer sits on the same DMA lane as its next writer -- keeps
# each DMA at <=2 sync waits (walrus HWDGE limit).
XBUFS = 8

_CACHE = {}


def _build_program(repeats=1):
    # Bacc (not raw Bass): its compile() splits multi-sem sync waits into
    # event semaphores -- TRN2 allows only 1 wait per instruction.
    # repeats>1 re-runs the streaming pass (for marginal-cost timing from
    # a bench harness); results are identical since every rep writes the
    # same values.
    nc = bacc.Bacc("TRN2", target_bir_lowering=False, debug=False,
                   num_devices=NCORES)

    x_d = nc.dram_tensor("x", [BL, C, H, W], F32, kind="ExternalInput")
    w1_d = nc.dram_tensor("w1p", [8, 128, MID], F32, kind="ExternalInput")
    b1_d = nc.dram_tensor("b1c", [MID, 1], F32, kind="ExternalInput")
    w2_d = nc.dram_tensor("w2c", [MID, E], F32, kind="ExternalInput")
    b2_d = nc.dram_tensor("b2r", [BL, E], F32, kind="ExternalInput")
    bn_d = nc.dram_tensor("bn", [BL, E], F32, kind="ExternalInput")
    io_d = nc.dram_tensor("iota8", [BL, E], F32, kind="ExternalInput")
    on_d = nc.dram_tensor("ones8", [E, 1], F32, kind="ExternalInput")
    out_d = nc.dram_tensor("out", [BL, 5], F32, kind="ExternalOutput")

    # [b, k, p, (h w)]: channel c = k*128 + p; 16 KiB contiguous per
    # partition, 2 MiB contiguous per (b, k) tile.
    xv = x_d.ap().rearrange("b (k p) h w -> b k p (h w)", k=CK, p=128)

    AF = mybir.ActivationFunctionType
    OP = mybir.AluOpType

    with tile.TileContext(nc) as tc, ExitStack() as ctx:
        xpool = ctx.enter_context(tc.tile_pool(name="xp", bufs=XBUFS))
        cpool = ctx.enter_context(tc.tile_pool(name="cp", bufs=1))
        ppool = ctx.enter_context(
            tc.tile_pool(name="pp", bufs=1, space=bass.MemorySpace.PSUM))

        # --- constants / small inputs (SWDGE so they don't perturb the
        # HWDGE lane rotation used by the big x loads) ---
        w1sb = cpool.tile([128, 8, MID], F32)
        nc.gpsimd.dma_start(w1sb[:], w1_d.ap().rearrange("k p m -> p k m"))
        b1sb = cpool.tile([MID, 1], F32)
        nc.gpsimd.dma_start(b1sb[:], b1_d.ap())
        w2sb = cpool.tile([MID, E], F32)
        nc.gpsimd.dma_start(w2sb[:], w2_d.ap())
        b2sb = cpool.tile([BL, E], F32)
        nc.gpsimd.dma_start(b2sb[:], b2_d.ap())
        bnsb = cpool.tile([BL, E], F32)
        nc.gpsimd.dma_start(bnsb[:], bn_d.ap())
        iosb = cpool.tile([BL, E], F32)
        nc.gpsimd.dma_start(iosb[:], io_d.ap())
        onsb = cpool.tile([E, 1], F32)
        nc.gpsimd.dma_start(onsb[:], on_d.ap())

        # --- global average pool (as sums; 1/4096 folded into W1) ---
        # gft[p, k*8 + b] = sum_s x[b, k*128 + p, s]
        gft = cpool.tile([128, 8 * BL], F32)
        for _rep in range(repeats):
            for b in range(BL):
                for k in range(CK):
                    xt = xpool.tile([128, S], F32)
                    nc.gpsimd.dma_start(xt[:], xv[b, k])
                    nc.vector.reduce_sum(
                        gft[:, k * BL + b:k * BL + b + 1],
                        xt[:],
                        axis=mybir.AxisListType.X,
                    )

        # --- router MLP ---
        # hT [MID, BL] = W1p.T @ gft  (accumulate over the 8 K-chunks of 128)
        psum_h = ppool.tile([MID, BL], F32)
        for k in range(8):
            nc.tensor.matmul(psum_h[:], w1sb[:, k, :],
                             gft[:, k * BL:(k + 1) * BL],
                             start=(k == 0), stop=(k == 7))
        # silu(z) = z * sigmoid(z), z = psum_h + b1  (Silu ACT table is not
        # implemented in the simulator, so build it from Sigmoid)
        hpre = cpool.tile([MID, BL], F32)
        nc.scalar.activation(hpre[:], psum_h[:], AF.Identity, bias=b1sb[:],
                             scale=1.0)
        hsig = cpool.tile([MID, BL], F32)
        nc.scalar.activation(hsig[:], hpre[:], AF.Sigmoid)
        hsb = cpool.tile([MID, BL], F32)
        nc.vector.tensor_tensor(hsb[:], hpre[:], hsig[:], op=OP.mult)

        # logits [BL, E] = (hT).T @ W2
        psum_l = ppool.tile([BL, E], F32)
        nc.tensor.matmul(psum_l[:], hsb[:], w2sb[:], start=True, stop=True)

        clean = cpool.tile([BL, E], F32)
        nc.vector.tensor_tensor(clean[:], psum_l[:], b2sb[:], op=OP.add)
        noisy = cpool.tile([BL, E], F32)
        nc.vector.tensor_tensor(noisy[:], psum_l[:], bnsb[:], op=OP.add)

        # --- top-2 ---
        m8 = cpool.tile([BL, 8], F32)
        nc.vector.max(m8[:], noisy[:])
        eq0 = cpool.tile([BL, E], F32)
        nc.vector.tensor_scalar(eq0[:], noisy[:], m8[:, 0:1], None,
                                op0=OP.is_equal)
        eq1 = cpool.tile([BL, E], F32)
        nc.vector.tensor_scalar(eq1[:], noisy[:], m8[:, 1:2], None,
                                op0=OP.is_equal)

        # (InstTensorTensorReduce crashes the exec unit on this runtime --
        # use separate mult + reduce_sum instead.)
        outsb = cpool.tile([BL, 5], F32)
        jj0 = cpool.tile([BL, E], F32)
        nc.vector.tensor_tensor(jj0[:], eq0[:], iosb[:], op=OP.mult)
        nc.vector.reduce_sum(outsb[:, 2:3], jj0[:], axis=mybir.AxisListType.X)
        jj1 = cpool.tile([BL, E], F32)
        nc.vector.tensor_tensor(jj1[:], eq1[:], iosb[:], op=OP.mult)
        nc.vector.reduce_sum(outsb[:, 3:4], jj1[:], axis=mybir.AxisListType.X)

        l0 = cpool.tile([BL, 1], F32)
        jj2 = cpool.tile([BL, E], F32)
        nc.vector.tensor_tensor(jj2[:], eq0[:], clean[:], op=OP.mult)
        nc.vector.reduce_sum(l0[:], jj2[:], axis=mybir.AxisListType.X)
        l1 = cpool.tile([BL, 1], F32)
        jj3 = cpool.tile([BL, E], F32)
        nc.vector.tensor_tensor(jj3[:], eq1[:], clean[:], op=OP.mult)
        nc.vector.reduce_sum(l1[:], jj3[:], axis=mybir.AxisListType.X)

        # --- expert counts: onehot.T @ ones ---
        onehot = cpool.tile([BL, E], F32)
        nc.vector.tensor_tensor(onehot[:], eq0[:], eq1[:], op=OP.add)
        psum_c = ppool.tile([E, 1], F32)
        nc.tensor.matmul(psum_c[:], onehot[:], onsb[:], start=True, stop=True)

        # --- softmax over the two selected clean logits ---
        mx = cpool.tile([BL, 1], F32)
        nc.vector.tensor_tensor(mx[:], l0[:], l1[:], op=OP.max)
        nmx = cpool.tile([BL, 1], F32)
        nc.vector.tensor_scalar_mul(nmx[:], mx[:], -1.0)
        ex0 = cpool.tile([BL, 1], F32)
        nc.scalar.activation(ex0[:], l0[:], AF.Exp, bias=nmx[:], scale=1.0)
        ex1 = cpool.tile([BL, 1], F32)
        nc.scalar.activation(ex1[:], l1[:], AF.Exp, bias=nmx[:], scale=1.0)
        ssum = cpool.tile([BL, 1], F32)
        nc.vector.tensor_tensor(ssum[:], ex0[:], ex1[:], op=OP.add)
        rcp = cpool.tile([BL, 1], F32)
        nc.vector.reciprocal(rcp[:], ssum[:])
        nc.vector.tensor_tensor(outsb[:, 0:1], ex0[:], rcp[:], op=OP.mult)
        nc.vector.tensor_tensor(outsb[:, 1:2], ex1[:], rcp[:], op=OP.mult)
        # DVE (not ACT) so the out DMA only waits on one compute engine.
        nc.vector.tensor_copy(outsb[:, 4:5], psum_c[:])

        nc.gpsimd.dma_start(out_d.ap(), outsb[:])

    nc.compile()
    return nc


def _get_program():
    if "nc" not in _CACHE:
        _CACHE["nc"] = _build_program()
    return _CACHE["nc"]


def _host_inputs(x, noise, W1, b1, W2, b2):
    x = np.ascontiguousarray(np.asarray(x, dtype=np.float32))
    noise = np.asarray(noise, dtype=np.float32)
    W1 = np.asarray(W1, dtype=np.float32)
    b1 = np.asarray(b1, dtype=np.float32)
    W2 = np.asarray(W2, dtype=np.float32)
    b2 = np.asarray(b2, dtype=np.float32)

    # W1 split into the 8 K-chunks of 128 channels, pre-scaled by 1/S to
    # turn spatial sums into means.
    w1p = np.ascontiguousarray((W1 / np.float32(S)).reshape(8, 128, MID))

    b1c = np.ascontiguousarray(b1.reshape(MID, 1))
    w2c = np.ascontiguousarray(W2)
    b2r = np.ascontiguousarray(np.broadcast_to(b2, (BL, E)))
    iota8 = np.ascontiguousarray(
        np.broadcast_to(np.arange(E, dtype=np.float32), (BL, E)))
    ones8 = np.ones((E, 1), dtype=np.float32)

    in_maps = []
    for c in range(NCORES):
        sl = slice(c * BL, (c + 1) * BL)
        in_maps.append({
            "x": x[sl],
            "w1p": w1p,
            "b1c": b1c,
            "w2c": w2c,
            "b2r": b2r,
            "bn": np.ascontiguousarray(b2r + 2.0 * noise[sl]),
            "iota8": iota8,
            "ones8": ones8,
        })
    return in_maps


def run(x, noise, W1, b1, W2, b2, trace=False, **spmd_kwargs):
    """Run the device kernel; returns (weights, indices, counts) plus the
    BassKernelResults (for profiling from a test harness)."""
    nc = _get_program()
    in_maps = _host_inputs(x, noise, W1, b1, W2, b2)
    res = run_bass_kernel_spmd(nc, in_maps, list(range(NCORES)),
                               trace=trace, **spmd_kwargs)
    outs = [res.results[c]["out"] for c in range(NCORES)]
    weights = np.concatenate([o[:, 0:2] for o in outs], axis=0)
    weights = np.ascontiguousarray(weights, dtype=np.float32)
    indices = np.rint(
        np.concatenate([o[:, 2:4] for o in outs], axis=0)).astype(np.int32)
    counts = np.rint(
        np.sum([o[:, 4] for o in outs], axis=0)).astype(np.int32)
    return (weights, indices, counts), res


def kernel(x, noise, W1, b1, W2, b2):
    (weights, indices, counts), _ = run(x, noise, W1, b1, W2, b2)
    return weights, indices, counts
